# revision 38
# baseline (speedup 1.0000x reference)
"""ALiBi multi-head attention on 8 TRN2 NeuronCores.

Sharding: core (b, g) = batch b in {0,1} x head-group g in {0..3}.  Host
permutes heads so core (b, g) holds global heads [g, g+4, g+8, g+12] —
one per ALiBi slope quartile — giving every core an identical banded
workload (SPMD) and balanced totals.  Each core projects its batch's
q/k/v through the column slice of wq/wk/wv for its heads, computes
banded-causal ALiBi attention, applies the row slice of wo, and writes a
partial [T, D] output.  Host sums the 4 partials per batch and adds bo.

Device-side layout trick: the host feeds qT/kT/vT (transposed) so every
matmul is a natural `lhsT.T @ rhs` with no on-device transposes:
  QT = wqT.T @ qT                          (wq pre-scaled by 1/sqrt(dk))
  scoresT[k,q] = KT_tile.T @ QT            (k on partitions)
  p = exp(scoresT) * exp(bias).T           (host precomputes exp(bias):
                                            no max pass; masked = exact 0)
  ctxT|denom = [V|1x64].T @ p              (denominator emitted broadcast
                                            across 64 partitions)
  out = ctxT.T @ woT_g                     (accumulated over head pairs)
Head pairs (2mp, 2mp+1) occupy PE row-groups 0-1/2-3 so their K=64
scores matmuls overlap; k-tiles are processed in pairs sharing a 2-bank
PSUM tile so exp and the exp(bias) multiply run as [128,1024] ops.
"""

import math
import os
import sys

import numpy as np

for _p in ("/opt/trn_rl_repo",):
    if os.path.isdir(_p) and _p not in sys.path:
        sys.path.insert(0, _p)

import ml_dtypes  # noqa: E402

import concourse.bass as bass  # noqa: E402
import concourse.mybir as mybir  # noqa: E402
import concourse.tile as tile  # noqa: E402
from concourse import bacc  # noqa: E402
from concourse.bass_utils import run_bass_kernel_spmd  # noqa: E402

BF16 = ml_dtypes.bfloat16

B, T, D, H = 2, 2048, 1024, 16
NCORES = 8
GH = 4            # heads per core
DK = D // H       # 64
GD = GH * DK      # 256 features per head group
P = 128
QC = 512          # q free-dim chunk
NQC = T // QC     # 4
NKT = T // P      # 16 k tiles
KT = D // P       # 8 contraction tiles for projections

_NC_CACHE = None
LAST_RESULT = None

# ALiBi band truncation.  Core slot s holds a head from slope-quartile s;
# slot s only needs the last NB[s] k-tiles per q-chunk (steeper slopes:
# exp(bias) underflows beyond ~C/slope positions).  Validated: C=8 keeps
# rel err at 4.373e-3, identical to full causal attention (cliff at C~4).
NB = [6, 6, 8, 16]


def _build_nc():
    nc = bacc.Bacc()
    f32 = mybir.dt.float32
    bf16 = mybir.dt.bfloat16

    qT = nc.declare_dram_parameter("qT", [D, T], bf16, isOutput=False)
    kT = nc.declare_dram_parameter("kT", [D, T], bf16, isOutput=False)
    vT = nc.declare_dram_parameter("vT", [D, T], bf16, isOutput=False)
    wqT = nc.declare_dram_parameter("wqT", [D, GD], bf16, isOutput=False)
    wkT = nc.declare_dram_parameter("wkT", [D, GD], bf16, isOutput=False)
    wvT = nc.declare_dram_parameter("wvT", [D, GD], bf16, isOutput=False)
    woT = nc.declare_dram_parameter("woT", [GD, D], bf16, isOutput=False)
    # exp(bias) transposed and pre-tiled: [h, qc, p, j, q] with k = j*128+p
    ebt = nc.declare_dram_parameter(
        "ebt", [GH, NQC, P, NKT, QC], bf16, isOutput=False
    )
    out = nc.declare_dram_parameter("out", [T, D], f32, isOutput=True)

    with tile.TileContext(nc) as tc:
        with (
            tc.tile_pool(name="weights", bufs=1) as wpool,
            tc.tile_pool(name="resid", bufs=1) as resid,
            tc.tile_pool(name="slab", bufs=3) as slab,
            tc.tile_pool(name="small", bufs=4) as spool,
            tc.tile_pool(name="ctxp", bufs=2) as cpool,
            tc.tile_pool(name="ps", bufs=2, space="PSUM") as pspool,
            tc.tile_pool(name="psc", bufs=2, space="PSUM") as psctx,
            tc.tile_pool(name="pso", bufs=2, space="PSUM") as psout,
        ):
            # ---- weights resident in SBUF -----------------------------
            # Queue order matters (sync queue is FIFO): wq first, then the
            # first two qT chunks, then the remaining weights — so the
            # first projection matmul isn't stuck behind 1.75 MB of
            # weights + a monolithic 4 MB slab transfer.
            wq_sb = wpool.tile([P, KT, GD], bf16, tag="wq")
            nc.sync.dma_start(out=wq_sb, in_=wqT[:].rearrange("(k p) m -> p k m", p=P))
            wk_sb = wpool.tile([P, KT, GD], bf16, tag="wk")
            wv_sb = wpool.tile([P, KT, GD], bf16, tag="wv")
            wo_sb = wpool.tile([P, 2, D], bf16, tag="wo")

            QT_sb = resid.tile([P, 2, T], bf16, tag="QT")
            KT_sb = resid.tile([P, 2, T], bf16, tag="KT")
            # V augmented with 64 ones-columns: the PV matmul then emits
            # [ctxT ; denom broadcast across 64 partitions] in one shot.
            Vaug = resid.tile([P, GH, NKT, 2 * DK], bf16, tag="Vaug")
            nc.vector.memset(Vaug[:, :, :, DK : 2 * DK], 1.0)

            # ---- phase A: projections --------------------------------
            first = True
            for xTd, w_sb, dst in ((qT, wq_sb, QT_sb), (kT, wk_sb, KT_sb)):
                xs = slab.tile([P, KT, T], bf16, tag="slab")
                for s4 in range(4):  # per-QC chunks: compute starts sooner
                    nc.sync.dma_start(
                        out=xs[:, :, s4 * QC : (s4 + 1) * QC],
                        in_=xTd[:].rearrange("(k p) t -> p k t", p=P)[
                            :, :, s4 * QC : (s4 + 1) * QC
                        ],
                    )
                    if first and s4 == 1:
                        nc.sync.dma_start(
                            out=wk_sb,
                            in_=wkT[:].rearrange("(k p) m -> p k m", p=P),
                        )
                        nc.sync.dma_start(
                            out=wv_sb,
                            in_=wvT[:].rearrange("(k p) m -> p k m", p=P),
                        )
                        nc.sync.dma_start(
                            out=wo_sb,
                            in_=woT[:].rearrange("(c p) e -> p c e", p=P),
                        )
                        first = False
                for qh in range(2):  # pair of 512-chunks -> one 1024 cast
                    for m in range(2):
                        ps = pspool.tile([P, 2, QC], mybir.dt.float32, tag="ps")
                        for s in range(2):
                            qc = 2 * qh + s
                            for k in range(KT):
                                nc.tensor.matmul(
                                    ps[:, s, :],
                                    w_sb[:, k, m * P : (m + 1) * P],
                                    xs[:, k, qc * QC : (qc + 1) * QC],
                                    start=(k == 0),
                                    stop=(k == KT - 1),
                                )
                        nc.vector.tensor_copy(
                            dst[:, m, qh * 2 * QC : (qh + 1) * 2 * QC],
                            ps[:].rearrange("p s q -> p (s q)"),
                        )

            vs = slab.tile([P, KT, T], bf16, tag="slab")
            for s4 in range(4):
                nc.sync.dma_start(
                    out=vs[:, :, s4 * QC : (s4 + 1) * QC],
                    in_=vT[:].rearrange("(k p) t -> p k t", p=P)[
                        :, :, s4 * QC : (s4 + 1) * QC
                    ],
                )
            for tp in range(NKT // 2):
                # [P, 2, QC] so each 256-wide group starts bank-aligned
                ps = pspool.tile([P, 2, QC], mybir.dt.float32, tag="ps")
                for s in range(2):
                    tt = 2 * tp + s
                    for k in range(KT):
                        nc.tensor.matmul(
                            ps[:, s, 0:GD],
                            vs[:, k, tt * P : (tt + 1) * P],
                            wv_sb[:, k, :],
                            start=(k == 0),
                            stop=(k == KT - 1),
                        )
                nc.vector.tensor_copy(
                    Vaug[:, :, 2 * tp : 2 * tp + 2, 0:DK],
                    ps[:, :, 0:GD].rearrange("p s (h d) -> p h s d", h=GH),
                )

            # ---- phase B: attention + output projection --------------
            for qc in range(NQC):
                nj = 4 * qc + 4  # causal: k tiles 0..4*qc+3 (always even)
                ctxT = cpool.tile([P, 2, QC], bf16, tag="ctxT")
                for mp in range(2):
                    ebs = []
                    pscs = []
                    jlos = []
                    for hloc in range(2):
                        jlo = max(0, nj - NB[2 * mp + hloc])
                        jlos.append(jlo)
                        eb = slab.tile([P, NKT, QC], bf16, tag="slab")
                        nc.sync.dma_start(
                            out=eb[:, jlo:nj, :],
                            in_=ebt[2 * mp + hloc, qc, :, jlo:nj, :],
                        )
                        ebs.append(eb)
                        pscs.append(
                            psctx.tile(
                                [2 * DK, QC],
                                mybir.dt.float32,
                                tag="psc",
                                name=f"psc{hloc}",
                            )
                        )
                    for jp in range((nj - min(jlos)) // 2):
                        for hloc in range(2):
                            j0 = jlos[hloc] + 2 * jp
                            if j0 >= nj:
                                continue
                            hp = hloc * DK
                            pss = pspool.tile(
                                [P, 2, QC], mybir.dt.float32, tag="ps"
                            )
                            for s in range(2):
                                j = j0 + s
                                nc.tensor.matmul(
                                    pss[:, s, :],
                                    KT_sb[hp : hp + DK, mp, j * P : (j + 1) * P],
                                    QT_sb[
                                        hp : hp + DK,
                                        mp,
                                        qc * QC : (qc + 1) * QC,
                                    ],
                                    start=True,
                                    stop=True,
                                )
                            ex = spool.tile([P, 2, QC], mybir.dt.bfloat16, tag="ex")
                            nc.scalar.activation(
                                ex, pss, mybir.ActivationFunctionType.Exp
                            )
                            pt = spool.tile([P, 2, QC], mybir.dt.bfloat16, tag="pt")
                            nc.vector.tensor_mul(
                                pt, ex, ebs[hloc][:, j0 : j0 + 2, :]
                            )
                            for s in range(2):
                                j = j0 + s
                                nc.tensor.matmul(
                                    pscs[hloc],
                                    Vaug[:, 2 * mp + hloc, j, :],
                                    pt[:, s, :],
                                    start=(j == jlos[hloc]),
                                    stop=(j == nj - 1),
                                )
                    for hloc in range(2):
                        hp = hloc * DK
                        # stage denom to SBUF (ScalarE; custom DVE recip can't
                        # read PSUM), then fast approximate reciprocal
                        den = spool.tile([DK, QC], mybir.dt.float32, tag="den")
                        nc.scalar.activation(
                            den,
                            pscs[hloc][DK : 2 * DK, :],
                            mybir.ActivationFunctionType.Copy,
                        )
                        rc = spool.tile([DK, QC], mybir.dt.float32, tag="rc")
                        nc.vector.reciprocal_approx_fast(rc, den)
                        nc.vector.tensor_mul(
                            ctxT[hp : hp + DK, mp, :],
                            pscs[hloc][0:DK, :],
                            rc,
                        )
                for q4 in range(4):
                    for ec in range(2):
                        po = psout.tile([P, QC], mybir.dt.float32, tag="po")
                        for c in range(2):
                            nc.tensor.matmul(
                                po,
                                ctxT[:, c, q4 * P : (q4 + 1) * P],
                                wo_sb[:, c, ec * QC : (ec + 1) * QC],
                                start=(c == 0),
                                stop=(c == 1),
                            )
                        ot = spool.tile([P, QC], mybir.dt.float32, tag="ot")
                        nc.vector.tensor_copy(ot, po)
                        r0 = qc * QC + q4 * P
                        nc.sync.dma_start(
                            out=out[r0 : r0 + P, ec * QC : (ec + 1) * QC], in_=ot
                        )
    nc.compile()
    return nc


def _get_nc():
    global _NC_CACHE
    if _NC_CACHE is None:
        _NC_CACHE = _build_nc()
    return _NC_CACHE


def _install_ntff_shim():
    """The agent image's antenv package lacks axon_hooks, so trn_boot's
    NTFF profile hook degraded silently.  Recreate the module and install
    the ctypes-based hook so trace=True yields exec_time_ns."""
    import types

    try:
        from antenv.axon_hooks import get_axon_ntff_profile_hook

        if get_axon_ntff_profile_hook() is not None:
            return
    except ImportError:
        pass

    import antenv

    mod = types.ModuleType("antenv.axon_hooks")
    _state = {"hook": None}

    def set_axon_ntff_profile_hook(h):
        _state["hook"] = h

    def get_axon_ntff_profile_hook():
        return _state["hook"]

    mod.set_axon_ntff_profile_hook = set_axon_ntff_profile_hook
    mod.get_axon_ntff_profile_hook = get_axon_ntff_profile_hook
    sys.modules["antenv.axon_hooks"] = mod
    antenv.axon_hooks = mod

    if "/root/.axon_site" not in sys.path and os.path.isdir("/root/.axon_site"):
        sys.path.insert(0, "/root/.axon_site")
    from trn_agent_boot.trn_boot import _ntff_profile_via_ctypes

    hook = _ntff_profile_via_ctypes("/opt/axon/libaxon_pjrt.so")
    if hook is None:
        raise RuntimeError("libaxon_pjrt.so lacks axon_start_nrt_profile")
    set_axon_ntff_profile_hook(hook)


def kernel(**inputs):
    global LAST_RESULT
    query = np.asarray(inputs["query"], np.float32)
    key = np.asarray(inputs["key"], np.float32)
    value = np.asarray(inputs["value"], np.float32)
    bias = np.asarray(inputs["alibi_bias"], np.float32)
    wq = np.asarray(inputs["wq"], np.float32)
    wk = np.asarray(inputs["wk"], np.float32)
    wv = np.asarray(inputs["wv"], np.float32)
    wo = np.asarray(inputs["wo"], np.float32)
    bo = np.asarray(inputs["bo"], np.float32)

    scale = 1.0 / math.sqrt(DK)
    with np.errstate(under="ignore", over="ignore"):
        eb = np.exp(bias)  # [H, T, T]; exp(-1e9) == 0 exactly

    # Core (b, g) holds heads [g, g+4, g+8, g+12] — one per slope quartile,
    # so every core's slot s has the same band NB[s] (SPMD) and total work
    # is balanced.  Tiled exp(bias).T: [GH, NQC, P, NKT, QC], k = j*128+p.
    ebt_g = []
    rows_g = []
    for g in range(4):
        hlist = [g, g + 4, g + 8, g + 12]
        rows_g.append(
            np.concatenate([np.arange(h * DK, (h + 1) * DK) for h in hlist])
        )
        tiles = []
        for hg in hlist:
            bT = np.ascontiguousarray(eb[hg].T)  # [k, q]
            tiles.append(
                bT.reshape(NKT, P, NQC, QC).transpose(2, 1, 0, 3)
            )
        ebt_g.append(np.stack(tiles, axis=0).astype(BF16))

    in_maps = []
    for b in range(B):
        qTb = np.ascontiguousarray(query[b].T).astype(BF16)  # [D, T]
        kTb = np.ascontiguousarray(key[b].T).astype(BF16)
        vTb = np.ascontiguousarray(value[b].T).astype(BF16)
        for g in range(4):
            rows = rows_g[g]
            in_maps.append(
                {
                    "qT": qTb,
                    "kT": kTb,
                    "vT": vTb,
                    "wqT": np.ascontiguousarray(
                        (wq[rows, :] * scale).T
                    ).astype(BF16),
                    "wkT": np.ascontiguousarray(wk[rows, :].T).astype(BF16),
                    "wvT": np.ascontiguousarray(wv[rows, :].T).astype(BF16),
                    "woT": np.ascontiguousarray(wo[:, rows].T).astype(BF16),
                    "ebt": ebt_g[g],
                }
            )

    nc = _get_nc()
    trace = os.environ.get("BASS_KERNEL_TRACE", "0") == "1"
    kwargs = {}
    if trace:
        try:
            _install_ntff_shim()
            kwargs["trace"] = True
            tc_env = os.environ.get("BASS_KERNEL_TRACE_CORES", "0")
            kwargs["trace_cores"] = [int(x) for x in tc_env.split(",")]
        except Exception as e:  # profiling is best-effort
            print(f"ntff shim failed ({e}); running without trace")
    res = run_bass_kernel_spmd(nc, in_maps, core_ids=list(range(NCORES)), **kwargs)
    LAST_RESULT = res

    final = np.zeros((B, T, D), np.float32)
    for b in range(B):
        acc = np.zeros((T, D), np.float32)
        for g in range(4):
            acc += np.asarray(res.results[b * 4 + g]["out"], np.float32)
        final[b] = acc + bo[None, :]
    return final


# revision 40
# speedup vs baseline: 1.0316x; 1.0316x over previous
"""ALiBi multi-head attention on 8 TRN2 NeuronCores.

Sharding: core (b, g) = batch b in {0,1} x head-group g in {0..3}.  Host
permutes heads so core (b, g) holds global heads [g, g+4, g+8, g+12] —
one per ALiBi slope quartile — giving every core an identical banded
workload (SPMD) and balanced totals.  Each core projects its batch's
q/k/v through the column slice of wq/wk/wv for its heads, computes
banded-causal ALiBi attention, applies the row slice of wo, and writes a
partial [T, D] output.  Host sums the 4 partials per batch and adds bo.

Device-side layout trick: the host feeds qT/kT/vT (transposed) so every
matmul is a natural `lhsT.T @ rhs` with no on-device transposes:
  QT = wqT.T @ qT                          (wq pre-scaled by 1/sqrt(dk))
  scoresT[k,q] = KT_tile.T @ QT            (k on partitions)
  p = exp(scoresT) * exp(bias).T           (host precomputes exp(bias):
                                            no max pass; masked = exact 0)
  ctxT|denom = [V|1x64].T @ p              (denominator emitted broadcast
                                            across 64 partitions)
  out = ctxT.T @ woT_g                     (accumulated over head pairs)
Head pairs (2mp, 2mp+1) occupy PE row-groups 0-1/2-3 so their K=64
scores matmuls overlap; k-tiles are processed in pairs sharing a 2-bank
PSUM tile so exp and the exp(bias) multiply run as [128,1024] ops.
"""

import math
import os
import sys

import numpy as np

for _p in ("/opt/trn_rl_repo",):
    if os.path.isdir(_p) and _p not in sys.path:
        sys.path.insert(0, _p)

import ml_dtypes  # noqa: E402

import concourse.bass as bass  # noqa: E402
import concourse.mybir as mybir  # noqa: E402
import concourse.tile as tile  # noqa: E402
from concourse import bacc  # noqa: E402
from concourse.bass_utils import run_bass_kernel_spmd  # noqa: E402

BF16 = ml_dtypes.bfloat16

B, T, D, H = 2, 2048, 1024, 16
NCORES = 8
GH = 4            # heads per core
DK = D // H       # 64
GD = GH * DK      # 256 features per head group
P = 128
QC = 512          # q free-dim chunk
NQC = T // QC     # 4
NKT = T // P      # 16 k tiles
KT = D // P       # 8 contraction tiles for projections

_NC_CACHE = None
LAST_RESULT = None

# ALiBi band truncation.  Core slot s holds a head from slope-quartile s;
# slot s only needs the last NB[s] k-tiles per q-chunk (steeper slopes:
# exp(bias) underflows beyond ~C/slope positions).  Validated: C=8 keeps
# rel err at 4.373e-3, identical to full causal attention (cliff at C~4).
NB = [6, 6, 8, 16]


def _build_nc():
    nc = bacc.Bacc()
    f32 = mybir.dt.float32
    bf16 = mybir.dt.bfloat16

    qT = nc.declare_dram_parameter("qT", [D, T], bf16, isOutput=False)
    kT = nc.declare_dram_parameter("kT", [D, T], bf16, isOutput=False)
    vT = nc.declare_dram_parameter("vT", [D, T], bf16, isOutput=False)
    wqT = nc.declare_dram_parameter("wqT", [D, GD], bf16, isOutput=False)
    wkT = nc.declare_dram_parameter("wkT", [D, GD], bf16, isOutput=False)
    wvT = nc.declare_dram_parameter("wvT", [D, GD], bf16, isOutput=False)
    woT = nc.declare_dram_parameter("woT", [GD, D], bf16, isOutput=False)
    # exp(bias) transposed and pre-tiled: [h, qc, p, j, q] with k = j*128+p
    ebt = nc.declare_dram_parameter(
        "ebt", [GH, NQC, P, NKT, QC], bf16, isOutput=False
    )
    out = nc.declare_dram_parameter("out", [T, D], f32, isOutput=True)

    with tile.TileContext(nc) as tc:
        with (
            tc.tile_pool(name="weights", bufs=1) as wpool,
            tc.tile_pool(name="resid", bufs=1) as resid,
            tc.tile_pool(name="slab", bufs=3) as slab,
            tc.tile_pool(name="small", bufs=4) as spool,
            tc.tile_pool(name="ctxp", bufs=2) as cpool,
            tc.tile_pool(name="ps", bufs=2, space="PSUM") as pspool,
            tc.tile_pool(name="psc", bufs=2, space="PSUM") as psctx,
            tc.tile_pool(name="pso", bufs=2, space="PSUM") as psout,
        ):
            # ---- weights resident in SBUF -----------------------------
            # Queue order matters (sync queue is FIFO): wq first, then the
            # first two qT chunks, then the remaining weights — so the
            # first projection matmul isn't stuck behind 1.75 MB of
            # weights + a monolithic 4 MB slab transfer.
            wq_sb = wpool.tile([P, KT, GD], bf16, tag="wq")
            nc.sync.dma_start(out=wq_sb, in_=wqT[:].rearrange("(k p) m -> p k m", p=P))
            wk_sb = wpool.tile([P, KT, GD], bf16, tag="wk")
            wv_sb = wpool.tile([P, KT, GD], bf16, tag="wv")
            wo_sb = wpool.tile([P, 2, D], bf16, tag="wo")

            QT_sb = resid.tile([P, 2, T], bf16, tag="QT")
            KT_sb = resid.tile([P, 2, T], bf16, tag="KT")
            # V augmented with 64 ones-columns: the PV matmul then emits
            # [ctxT ; denom broadcast across 64 partitions] in one shot.
            Vaug = resid.tile([P, GH, NKT, 2 * DK], bf16, tag="Vaug")
            nc.vector.memset(Vaug[:, :, :, DK : 2 * DK], 1.0)

            # ---- phase A: projections --------------------------------
            first = True
            for xTd, w_sb, dst in ((qT, wq_sb, QT_sb), (kT, wk_sb, KT_sb)):
                xs = slab.tile([P, KT, T], bf16, tag="slab")
                # chunk along kt: 4KB-contiguous bursts per partition, and
                # the k=0 matmuls start as soon as the first chunk lands
                for k2 in range(4):
                    nc.sync.dma_start(
                        out=xs[:, 2 * k2 : 2 * k2 + 2, :],
                        in_=xTd[:].rearrange("(k p) t -> p k t", p=P)[
                            :, 2 * k2 : 2 * k2 + 2, :
                        ],
                    )
                    if first and k2 == 1:
                        nc.sync.dma_start(
                            out=wk_sb,
                            in_=wkT[:].rearrange("(k p) m -> p k m", p=P),
                        )
                        nc.sync.dma_start(
                            out=wv_sb,
                            in_=wvT[:].rearrange("(k p) m -> p k m", p=P),
                        )
                        nc.sync.dma_start(
                            out=wo_sb,
                            in_=woT[:].rearrange("(c p) e -> p c e", p=P),
                        )
                        first = False
                for qh in range(2):  # pair of 512-chunks -> one 1024 cast
                    for m in range(2):
                        ps = pspool.tile([P, 2, QC], mybir.dt.float32, tag="ps")
                        for s in range(2):
                            qc = 2 * qh + s
                            for k in range(KT):
                                nc.tensor.matmul(
                                    ps[:, s, :],
                                    w_sb[:, k, m * P : (m + 1) * P],
                                    xs[:, k, qc * QC : (qc + 1) * QC],
                                    start=(k == 0),
                                    stop=(k == KT - 1),
                                )
                        nc.vector.tensor_copy(
                            dst[:, m, qh * 2 * QC : (qh + 1) * 2 * QC],
                            ps[:].rearrange("p s q -> p (s q)"),
                        )

            vs = slab.tile([P, KT, T], bf16, tag="slab")
            for k2 in range(4):
                nc.sync.dma_start(
                    out=vs[:, 2 * k2 : 2 * k2 + 2, :],
                    in_=vT[:].rearrange("(k p) t -> p k t", p=P)[
                        :, 2 * k2 : 2 * k2 + 2, :
                    ],
                )
            for tp in range(NKT // 2):
                # [P, 2, QC] so each 256-wide group starts bank-aligned
                ps = pspool.tile([P, 2, QC], mybir.dt.float32, tag="ps")
                for s in range(2):
                    tt = 2 * tp + s
                    for k in range(KT):
                        nc.tensor.matmul(
                            ps[:, s, 0:GD],
                            vs[:, k, tt * P : (tt + 1) * P],
                            wv_sb[:, k, :],
                            start=(k == 0),
                            stop=(k == KT - 1),
                        )
                nc.vector.tensor_copy(
                    Vaug[:, :, 2 * tp : 2 * tp + 2, 0:DK],
                    ps[:, :, 0:GD].rearrange("p s (h d) -> p h s d", h=GH),
                )

            # ---- phase B: attention + output projection --------------
            for qc in range(NQC):
                nj = 4 * qc + 4  # causal: k tiles 0..4*qc+3 (always even)
                ctxT = cpool.tile([P, 2, QC], bf16, tag="ctxT")
                for mp in range(2):
                    ebs = []
                    pscs = []
                    jlos = []
                    for hloc in range(2):
                        jlo = max(0, nj - NB[2 * mp + hloc])
                        jlos.append(jlo)
                        eb = slab.tile([P, NKT, QC], bf16, tag="slab")
                        nc.sync.dma_start(
                            out=eb[:, jlo:nj, :],
                            in_=ebt[2 * mp + hloc, qc, :, jlo:nj, :],
                        )
                        ebs.append(eb)
                        pscs.append(
                            psctx.tile(
                                [2 * DK, QC],
                                mybir.dt.float32,
                                tag="psc",
                                name=f"psc{hloc}",
                            )
                        )
                    for jp in range((nj - min(jlos)) // 2):
                        for hloc in range(2):
                            j0 = jlos[hloc] + 2 * jp
                            if j0 >= nj:
                                continue
                            hp = hloc * DK
                            pss = pspool.tile(
                                [P, 2, QC], mybir.dt.float32, tag="ps"
                            )
                            for s in range(2):
                                j = j0 + s
                                nc.tensor.matmul(
                                    pss[:, s, :],
                                    KT_sb[hp : hp + DK, mp, j * P : (j + 1) * P],
                                    QT_sb[
                                        hp : hp + DK,
                                        mp,
                                        qc * QC : (qc + 1) * QC,
                                    ],
                                    start=True,
                                    stop=True,
                                )
                            ex = spool.tile([P, 2, QC], mybir.dt.bfloat16, tag="ex")
                            nc.scalar.activation(
                                ex, pss, mybir.ActivationFunctionType.Exp
                            )
                            pt = spool.tile([P, 2, QC], mybir.dt.bfloat16, tag="pt")
                            nc.vector.tensor_mul(
                                pt, ex, ebs[hloc][:, j0 : j0 + 2, :]
                            )
                            for s in range(2):
                                j = j0 + s
                                nc.tensor.matmul(
                                    pscs[hloc],
                                    Vaug[:, 2 * mp + hloc, j, :],
                                    pt[:, s, :],
                                    start=(j == jlos[hloc]),
                                    stop=(j == nj - 1),
                                )
                    for hloc in range(2):
                        hp = hloc * DK
                        # stage denom to SBUF (ScalarE; custom DVE recip can't
                        # read PSUM), then fast approximate reciprocal
                        den = spool.tile([DK, QC], mybir.dt.float32, tag="den")
                        nc.scalar.activation(
                            den,
                            pscs[hloc][DK : 2 * DK, :],
                            mybir.ActivationFunctionType.Copy,
                        )
                        rc = spool.tile([DK, QC], mybir.dt.float32, tag="rc")
                        nc.vector.reciprocal_approx_fast(rc, den)
                        nc.vector.tensor_mul(
                            ctxT[hp : hp + DK, mp, :],
                            pscs[hloc][0:DK, :],
                            rc,
                        )
                for q4 in range(4):
                    for ec in range(2):
                        po = psout.tile([P, QC], mybir.dt.float32, tag="po")
                        for c in range(2):
                            nc.tensor.matmul(
                                po,
                                ctxT[:, c, q4 * P : (q4 + 1) * P],
                                wo_sb[:, c, ec * QC : (ec + 1) * QC],
                                start=(c == 0),
                                stop=(c == 1),
                            )
                        ot = spool.tile([P, QC], mybir.dt.float32, tag="ot")
                        nc.vector.tensor_copy(ot, po)
                        r0 = qc * QC + q4 * P
                        nc.sync.dma_start(
                            out=out[r0 : r0 + P, ec * QC : (ec + 1) * QC], in_=ot
                        )
    nc.compile()
    return nc


def _get_nc():
    global _NC_CACHE
    if _NC_CACHE is None:
        _NC_CACHE = _build_nc()
    return _NC_CACHE


def _install_ntff_shim():
    """The agent image's antenv package lacks axon_hooks, so trn_boot's
    NTFF profile hook degraded silently.  Recreate the module and install
    the ctypes-based hook so trace=True yields exec_time_ns."""
    import types

    try:
        from antenv.axon_hooks import get_axon_ntff_profile_hook

        if get_axon_ntff_profile_hook() is not None:
            return
    except ImportError:
        pass

    import antenv

    mod = types.ModuleType("antenv.axon_hooks")
    _state = {"hook": None}

    def set_axon_ntff_profile_hook(h):
        _state["hook"] = h

    def get_axon_ntff_profile_hook():
        return _state["hook"]

    mod.set_axon_ntff_profile_hook = set_axon_ntff_profile_hook
    mod.get_axon_ntff_profile_hook = get_axon_ntff_profile_hook
    sys.modules["antenv.axon_hooks"] = mod
    antenv.axon_hooks = mod

    if "/root/.axon_site" not in sys.path and os.path.isdir("/root/.axon_site"):
        sys.path.insert(0, "/root/.axon_site")
    from trn_agent_boot.trn_boot import _ntff_profile_via_ctypes

    hook = _ntff_profile_via_ctypes("/opt/axon/libaxon_pjrt.so")
    if hook is None:
        raise RuntimeError("libaxon_pjrt.so lacks axon_start_nrt_profile")
    set_axon_ntff_profile_hook(hook)


def kernel(**inputs):
    global LAST_RESULT
    query = np.asarray(inputs["query"], np.float32)
    key = np.asarray(inputs["key"], np.float32)
    value = np.asarray(inputs["value"], np.float32)
    bias = np.asarray(inputs["alibi_bias"], np.float32)
    wq = np.asarray(inputs["wq"], np.float32)
    wk = np.asarray(inputs["wk"], np.float32)
    wv = np.asarray(inputs["wv"], np.float32)
    wo = np.asarray(inputs["wo"], np.float32)
    bo = np.asarray(inputs["bo"], np.float32)

    scale = 1.0 / math.sqrt(DK)
    with np.errstate(under="ignore", over="ignore"):
        eb = np.exp(bias)  # [H, T, T]; exp(-1e9) == 0 exactly

    # Core (b, g) holds heads [g, g+4, g+8, g+12] — one per slope quartile,
    # so every core's slot s has the same band NB[s] (SPMD) and total work
    # is balanced.  Tiled exp(bias).T: [GH, NQC, P, NKT, QC], k = j*128+p.
    ebt_g = []
    rows_g = []
    for g in range(4):
        hlist = [g, g + 4, g + 8, g + 12]
        rows_g.append(
            np.concatenate([np.arange(h * DK, (h + 1) * DK) for h in hlist])
        )
        tiles = []
        for hg in hlist:
            bT = np.ascontiguousarray(eb[hg].T)  # [k, q]
            tiles.append(
                bT.reshape(NKT, P, NQC, QC).transpose(2, 1, 0, 3)
            )
        ebt_g.append(np.stack(tiles, axis=0).astype(BF16))

    in_maps = []
    for b in range(B):
        qTb = np.ascontiguousarray(query[b].T).astype(BF16)  # [D, T]
        kTb = np.ascontiguousarray(key[b].T).astype(BF16)
        vTb = np.ascontiguousarray(value[b].T).astype(BF16)
        for g in range(4):
            rows = rows_g[g]
            in_maps.append(
                {
                    "qT": qTb,
                    "kT": kTb,
                    "vT": vTb,
                    "wqT": np.ascontiguousarray(
                        (wq[rows, :] * scale).T
                    ).astype(BF16),
                    "wkT": np.ascontiguousarray(wk[rows, :].T).astype(BF16),
                    "wvT": np.ascontiguousarray(wv[rows, :].T).astype(BF16),
                    "woT": np.ascontiguousarray(wo[:, rows].T).astype(BF16),
                    "ebt": ebt_g[g],
                }
            )

    nc = _get_nc()
    trace = os.environ.get("BASS_KERNEL_TRACE", "0") == "1"
    kwargs = {}
    if trace:
        try:
            _install_ntff_shim()
            kwargs["trace"] = True
            tc_env = os.environ.get("BASS_KERNEL_TRACE_CORES", "0")
            kwargs["trace_cores"] = [int(x) for x in tc_env.split(",")]
        except Exception as e:  # profiling is best-effort
            print(f"ntff shim failed ({e}); running without trace")
    res = run_bass_kernel_spmd(nc, in_maps, core_ids=list(range(NCORES)), **kwargs)
    LAST_RESULT = res

    final = np.zeros((B, T, D), np.float32)
    for b in range(B):
        acc = np.zeros((T, D), np.float32)
        for g in range(4):
            acc += np.asarray(res.results[b * 4 + g]["out"], np.float32)
        final[b] = acc + bo[None, :]
    return final
